# revision 7
# baseline (speedup 1.0000x reference)
"""TRN2 Bass kernel for nn_Block1_43542378447225 (v2: bf16 + short chain).

Per sample (one NeuronCore; batch=2 -> cores 0/1 do real work):
  conv1 -> a1p (padded bf16) -> conv2 directly from strided a1p views (16
  chained matmuls) -> z2 -> Hopfield#1 in S^T layout (m on partitions, no
  transposes; softmax denominator via a ones-column in KV; normalization by
  reciprocal + ones-stationary broadcast matmul) -> Dm2 -> backward -> C ->
  round C to bf16 (keeps the argmin compare exact through the bf16
  permutation matmuls) -> 9 shifted candidate matmuls into zero-prefilled
  PSUM banks -> min-reduce -> eW unpermute -> mask -> masked forward ->
  Hopfield#2 unnormalized; host divides by the denominator row.

Layouts:
  pq = p*8+q (64 outputs), uv = u*10+v (100 composite offsets),
  kc = a*32+c1 within conv2-kernel-column chunk t (hidden dim 4x128).
"""
import numpy as np
import ml_dtypes

import concourse.bass as bass
import concourse.bacc as bacc
import concourse.mybir as mybir
import concourse.tile as tile
from concourse.bass_utils import run_bass_kernel_spmd

F32 = mybir.dt.float32
BF16 = mybir.dt.bfloat16
AF = mybir.ActivationFunctionType
ALU = mybir.AluOpType
BF = ml_dtypes.bfloat16

N_CORES = 8
BETA = 0.125  # 1/sqrt(64)

_CACHE = {}


# ---------------------------------------------------------------- host prep
def _host_prep(w1, b1, w2, b2, K, Vw):
    # wE [128, 833]: w2fT | KT | Vw | b2
    wE = np.zeros((128, 833), np.float32)
    # w2fT[a*32+c1, t*64+o2] = w2[o2, c1, t, a]
    wE[:, 0:256] = np.transpose(w2, (3, 1, 2, 0)).reshape(128, 256)
    wE[0:64, 256:768] = K.T                     # KT [c, m]
    wE[0:64, 768:832] = Vw
    wE[0:64, 832:833] = b2[:, None]

    # wB [128, 912]: w2b (2x folded) | Scomb
    wB = np.zeros((128, 912), np.float32)
    wB[0:64, 0:512] = 2.0 * np.transpose(w2, (0, 2, 3, 1)).reshape(64, 512)
    w1s = w1.sum(axis=1)
    Scomb = np.zeros((4, 32, 4, 100), np.float32)  # [a, c1, t, uv]
    W1big = np.zeros((100, 3, 4, 128), np.float32)  # [uv, h, t, a*32+c1]
    for t in range(4):
        for a in range(4):
            for u in range(10):
                ki = u - 2 * t
                if not (0 <= ki < 4):
                    continue
                for v in range(10):
                    kj = v - 2 * a
                    if not (0 <= kj < 4):
                        continue
                    Scomb[a, :, t, u * 10 + v] = w1s[:, ki, kj]
                    W1big[u * 10 + v, :, t, a * 32:(a + 1) * 32] = \
                        w1[:, :, ki, kj].T
    wB[:, 512:912] = Scomb.reshape(128, 400)

    # CandK [uv, 9, 16] / PermB [16, 900]
    PermF = np.zeros((100, 9, 16), np.float32)
    for k in range(9):
        dp, dq = k // 3 - 1, k % 3 - 1
        for im in range(4):
            u = 4 * dp + im + 3
            if not (0 <= u < 10):
                continue
            for jm in range(4):
                v = 4 * dq + jm + 3
                if not (0 <= v < 10):
                    continue
                PermF[u * 10 + v, k, im * 4 + jm] = 1.0
    PermB = np.transpose(PermF, (2, 1, 0)).reshape(16, 900)

    # wC1 [100, 1044]: CandK | PermB   (X appended per-sample -> [100, 1236])
    wC1 = np.zeros((100, 1044), np.float32)
    wC1[:, 0:144] = PermF.reshape(100, 144)
    wC1[0:16, 144:1044] = PermB

    return {"wE": wE.astype(BF), "wB": wB.astype(BF),
            "wC2": np.ascontiguousarray(W1big.reshape(100, 1536)).astype(BF),
            "_wC1": wC1,
            "_w2ta": np.ascontiguousarray(
                np.transpose(w2, (1, 2, 3, 0)).reshape(32, 1024)),
            "_w1f": np.ascontiguousarray(
                np.transpose(w1, (2, 3, 1, 0)).reshape(48, 32)),
            "_b1": np.ascontiguousarray(b1[:, None])}


def _sample_prep(x_s, w1f, b1c, w2ta, wC1):
    xp1 = np.pad(x_s, ((0, 0), (1, 1), (1, 1)))
    xp3 = np.pad(x_s, ((0, 0), (3, 3), (3, 3)))
    P1 = np.zeros((4, 4, 3, 16, 16), np.float32)
    for kr in range(4):
        for ks in range(4):
            P1[kr, ks] = xp1[:, kr:kr + 32:2, ks:ks + 32:2][:, :16, :16]
    X = np.zeros((10, 10, 3, 8, 8), np.float32)
    for u in range(10):
        for v in range(10):
            X[u, v] = xp3[:, u:u + 32:4, v:v + 32:4][:, :8, :8]
    cv = np.zeros((48, 1313), np.float32)
    cv[:, 0:256] = P1.reshape(48, 256)
    cv[:, 256:288] = w1f
    cv[0:32, 288:289] = b1c
    cv[0:32, 289:1313] = w2ta
    wC1x = np.zeros((100, 1236), np.float32)
    wC1x[:, 0:1044] = wC1
    wC1x[:, 1044:1236] = X.reshape(100, 192)
    return cv.astype(BF), wC1x.astype(BF)


# ---------------------------------------------------------------- device build
def _build_nc(debug=False):
    nc = bacc.Bacc("TRN2", target_bir_lowering=False, debug=False,
                   num_devices=N_CORES)
    d_cv = nc.dram_tensor("cv", [48, 1313], BF16, kind="ExternalInput")
    d_wE = nc.dram_tensor("wE", [128, 833], BF16, kind="ExternalInput")
    d_wB = nc.dram_tensor("wB", [128, 912], BF16, kind="ExternalInput")
    d_wC1 = nc.dram_tensor("wC1", [100, 1236], BF16, kind="ExternalInput")
    d_wC2 = nc.dram_tensor("wC2", [100, 1536], BF16, kind="ExternalInput")
    out_t = nc.dram_tensor("out", [65, 64], F32, kind="ExternalOutput")
    probes = {}

    def probe(name, shape):
        if debug:
            probes[name] = nc.dram_tensor("probe_" + name, shape, F32,
                                          kind="ExternalOutput")
        return probes.get(name)

    with tile.TileContext(nc) as tc:
        with tc.tile_pool(name="sb", bufs=1) as sb, \
             tc.tile_pool(name="ps", bufs=1, space="PSUM") as ps:
            # ---- DMA issues first (SP: cv, wB, wC2, out / ACT: wE, wC1)
            cv = sb.tile([48, 1313], BF16, tag="cv")
            nc.sync.dma_start(out=cv[:], in_=d_cv[:])
            wB = sb.tile([128, 912], BF16, tag="wB")
            nc.sync.dma_start(out=wB[:], in_=d_wB[:])
            wC2 = sb.tile([100, 1536], BF16, tag="wC2")
            nc.sync.dma_start(out=wC2[:], in_=d_wC2[:])
            wE = sb.tile([128, 833], BF16, tag="wE")
            nc.scalar.dma_start(out=wE[:], in_=d_wE[:])
            wC1 = sb.tile([100, 1236], BF16, tag="wC1")
            nc.scalar.dma_start(out=wC1[:], in_=d_wC1[:])

            # ---- init (DVE + Pool), PE warm-up
            warm = sb.tile([2, 8], BF16, tag="warm")
            nc.vector.memset(warm[:], 0.0)
            ones_f = sb.tile([1, 64], F32, tag="ones_f")
            nc.vector.memset(ones_f[:], 1.0)
            KV = sb.tile([128, 4, 65], BF16, tag="KV")
            nc.vector.memset(KV[:, :, 64:65], 1.0)
            a1p = sb.tile([32, 18, 18], BF16, tag="a1p")
            nc.gpsimd.memset(a1p[:], 0.0)
            eB = sb.tile([16, 12, 8], BF16, tag="eB")
            nc.gpsimd.memset(eB[:], 0.0)
            C_pad = sb.tile([100, 10, 10], BF16, tag="C_pad")
            nc.gpsimd.memset(C_pad[:], 0.0)
            cstkA = ps.tile([16, 8, 8, 5], F32, tag="cstkA", bufs=1)
            nc.vector.memset(cstkA[:], 0.0)
            cstkB = ps.tile([16, 8, 8, 5], F32, tag="cstkB", bufs=1)
            nc.vector.memset(cstkB[:], 0.0)
            for w_ in range(3):
                warm_ps = ps.tile([8, 8], F32, tag="R", bufs=1,
                                  name=f"warm{w_}")
                nc.tensor.matmul(warm_ps[:], warm[0:2, :], warm[0:2, :],
                                 start=True, stop=True)

            # views
            P1 = cv[:, 0:256]
            w1f = cv[:, 256:288]
            b1 = cv[0:32, 288:289]
            KTc = [wE[0:64, 256 + t * 128:256 + (t + 1) * 128]
                   for t in range(4)]
            Vw = wE[0:64, 768:832]
            b2 = wE[0:64, 832:833]
            w2b = wB[0:64, 0:512]
            Scomb = wB[:, 512:912].rearrange("k (t u) -> k t u", t=4)
            CandK = wC1[0:100, 0:144]
            PermB = wC1[0:16, 144:1044]
            Xv = wC1[0:100, 1044:1236].rearrange("u (h q) -> u h q", h=3)
            W1big = wC2[0:100, :].rearrange("u (h t k) -> u h t k", h=3, t=4)

            # ---- conv1 + relu -> a1p [32, 18, 18] bf16 (borders pre-zeroed)
            a1_ps = ps.tile([32, 256], F32, tag="big", bufs=2)
            nc.tensor.matmul(a1_ps[:], w1f, P1, start=True, stop=True)
            nc.scalar.activation(
                out=a1p[:, 1:17, 1:17],
                in_=a1_ps[:].rearrange("c (p q) -> c p q", p=16),
                func=AF.Relu, bias=b1, scale=1.0)

            # ---- conv2 straight from strided a1p views (16 chained matmuls)
            a1p_ap = a1p[:]
            z2_ps = ps.tile([64, 64], F32, tag="sm", bufs=2)
            idx = 0
            for t in range(4):
                for a in range(4):
                    mov = bass.AP(tensor=a1p_ap.tensor,
                                  offset=a1p_ap.offset + t * 18 + a,
                                  ap=[[324, 32], [36, 8], [2, 8]])
                    stat = cv[0:32, 289 + (t * 4 + a) * 64:289 + (t * 4 + a + 1) * 64]
                    nc.tensor.matmul(z2_ps[:], stat, mov,
                                     start=(idx == 0), stop=(idx == 15))
                    idx += 1
            z2b = sb.tile([64, 64], BF16, tag="z2b")
            nc.scalar.activation(out=z2b[:], in_=z2_ps[:], func=AF.Relu,
                                 bias=b2, scale=1.0)
            if debug:
                nc.sync.dma_start(out=probe("z2", [64, 64])[:], in_=z2b[:])

            # ---- KV = K @ Vw chunks (PE), copies on Pool+DVE
            kv_ps = ps.tile([128, 256], F32, tag="kv", bufs=1)
            for t in range(4):
                nc.tensor.matmul(kv_ps[:, t * 64:(t + 1) * 64], KTc[t], Vw,
                                 start=True, stop=True)

            # ---- M1W masks (relu1 deriv in kc layout), split DVE/Pool
            M1W = sb.tile([128, 4, 64], BF16, tag="M1W")
            for a in range(4):
                src = bass.AP(tensor=a1p_ap.tensor,
                              offset=a1p_ap.offset + a,
                              ap=[[324, 32], [18, 4], [36, 8], [2, 8]])
                eng = nc.vector if a < 2 else nc.gpsimd
                eng.tensor_scalar(out=M1W[a * 32:(a + 1) * 32, :, :],
                                  in0=src, scalar1=0.0, scalar2=None,
                                  op0=ALU.not_equal)
            # KV copies (Pool x2, DVE x2)
            for t in range(4):
                eng = nc.gpsimd if t % 2 == 0 else nc.vector
                eng.tensor_copy(out=KV[:, t, 0:64],
                                in_=kv_ps[:, t * 64:(t + 1) * 64])
            m2 = sb.tile([64, 64], BF16, tag="m2")
            nc.vector.tensor_scalar(out=m2[:], in0=z2b[:], scalar1=0.0,
                                    scalar2=None, op0=ALU.not_equal)

            # ---- Hopfield #1: S^T (4 indep) -> exp -> q (4 chained w/ ones
            #      col) -> reciprocal -> broadcast matmul -> Dm2
            ST1 = ps.tile([128, 256], F32, tag="big", bufs=2)
            for t in range(4):
                nc.tensor.matmul(ST1[:, t * 64:(t + 1) * 64], KTc[t], z2b[:],
                                 start=True, stop=True)
            att1 = sb.tile([128, 256], BF16, tag="att1")
            nc.scalar.activation(out=att1[:], in_=ST1[:], func=AF.Exp,
                                 bias=0.0, scale=BETA)
            q1_ps = ps.tile([65, 64], F32, tag="sm", bufs=2)
            for t in range(4):
                nc.tensor.matmul(q1_ps[:], KV[:, t, :],
                                 att1[:, t * 64:(t + 1) * 64],
                                 start=(t == 0), stop=(t == 3))
            r1 = sb.tile([1, 64], F32, tag="r1")
            nc.vector.reciprocal(r1[:], q1_ps[64:65, :])
            R_ps = ps.tile([64, 64], F32, tag="R", bufs=1)
            nc.tensor.matmul(R_ps[:], ones_f[:], r1[:], start=True, stop=True)
            qR = sb.tile([64, 64], BF16, tag="qR")
            nc.vector.tensor_tensor(out=qR[:], in0=q1_ps[0:64, :], in1=R_ps[:],
                                    op=ALU.mult)
            Dm2a = sb.tile([64, 64], BF16, tag="Dm2a")
            nc.vector.tensor_tensor(out=Dm2a[:], in0=z2b[:], in1=qR[:],
                                    op=ALU.subtract)
            Dm2 = sb.tile([64, 64], BF16, tag="Dm2")
            nc.vector.tensor_tensor(out=Dm2[:], in0=Dm2a[:], in1=m2[:],
                                    op=ALU.mult)

            # ---- backward: g1 (4 indep into one PSUM), g1m, C, C_bf
            g1_ps = ps.tile([128, 256], F32, tag="big", bufs=2)
            for t in range(4):
                nc.tensor.matmul(g1_ps[:, t * 64:(t + 1) * 64],
                                 w2b[:, t * 128:(t + 1) * 128], Dm2[:],
                                 start=True, stop=True)
            g1m = sb.tile([128, 4, 64], BF16, tag="g1m")
            nc.vector.tensor_tensor(
                out=g1m[:].rearrange("k t q -> k (t q)"), in0=g1_ps[:],
                in1=M1W[:].rearrange("k t q -> k (t q)"), op=ALU.mult)
            C_ps = ps.tile([100, 64], F32, tag="sm", bufs=2)
            for t in range(4):
                nc.tensor.matmul(C_ps[:], Scomb[:, t, :], g1m[:, t, :],
                                 start=(t == 0), stop=(t == 3))
            nc.vector.tensor_copy(
                out=C_pad[:, 1:9, 1:9],
                in_=C_ps[:].rearrange("u (p q) -> u p q", p=8))
            if debug:
                C_dbg = sb.tile([100, 64], F32, tag="C_dbg")
                nc.vector.tensor_copy(
                    out=C_dbg[:].rearrange("u (p q) -> u p q", p=8),
                    in_=C_pad[:, 1:9, 1:9])
                nc.sync.dma_start(out=probe("C", [100, 64])[:], in_=C_dbg[:])

            # ---- e_min: 9 shifted matmuls into zero-prefilled PSUM banks,
            #      PSUM->SBUF copies (ACT || DVE), one bf16 min-reduce
            C_ap = C_pad[:]
            for k in range(9):
                dp, dq = k // 3 - 1, k % 3 - 1
                bank, s = (cstkA, k) if k < 4 else (cstkB, k - 4)
                mov = bass.AP(tensor=C_ap.tensor,
                              offset=C_ap.offset + (1 - dp) * 10 + (1 - dq),
                              ap=[[100, 100], [10, 8], [1, 8]])
                bank_ap = bank[:]
                outap = bass.AP(tensor=bank_ap.tensor,
                                offset=bank_ap.offset + s,
                                ap=[[320, 16], [5, 64]])
                nc.tensor.matmul(outap,
                                 wC1[0:100, k * 16:(k + 1) * 16], mov,
                                 start=True, stop=True)
            cstk_sb = sb.tile([16, 8, 8, 10], BF16, tag="cstk_sb")
            nc.scalar.copy(out=cstk_sb[:, :, :, 0:5], in_=cstkA[:])
            nc.vector.tensor_copy(out=cstk_sb[:, :, :, 5:10], in_=cstkB[:])
            nc.vector.tensor_reduce(out=eB[:, 2:10, :], in_=cstk_sb[:],
                                    axis=mybir.AxisListType.X, op=ALU.min)
            eBf = eB[:].rearrange("a b c -> a (b c)")
            eW_ps = ps.tile([100, 64], F32, tag="cstkA", bufs=1)
            for k in range(9):
                dp, dq = k // 3 - 1, k % 3 - 1
                off = 16 + 8 * dp + dq
                nc.tensor.matmul(eW_ps[:], PermB[:, k * 100:(k + 1) * 100],
                                 eBf[:, off:off + 64],
                                 start=(k == 0), stop=(k == 8))
            maskw = sb.tile([100, 64], BF16, tag="maskw")
            nc.vector.tensor_tensor(
                out=maskw[:].rearrange("u (p q) -> u p q", p=8),
                in0=C_pad[:, 1:9, 1:9],
                in1=eW_ps[:].rearrange("u (p q) -> u p q", p=8),
                op=ALU.is_le)
            if debug:
                mw_dbg = sb.tile([100, 64], F32, tag="mw_dbg")
                nc.vector.tensor_copy(out=mw_dbg[:], in_=maskw[:])
                nc.sync.dma_start(out=probe("maskw", [100, 64])[:],
                                  in_=mw_dbg[:])

            # ---- masked forward
            Xm = sb.tile([100, 3, 64], BF16, tag="Xm")
            mask_b = bass.AP(tensor=maskw[:].tensor, offset=maskw[:].offset,
                             ap=[[64, 100], [0, 3], [1, 64]])
            nc.vector.tensor_tensor(out=Xm[:], in0=Xv, in1=mask_b, op=ALU.mult)
            u1_ps = ps.tile([128, 256], F32, tag="big", bufs=2)
            for t in range(4):
                for h in range(3):
                    nc.tensor.matmul(u1_ps[:, t * 64:(t + 1) * 64],
                                     W1big[:, h, t, :], Xm[:, h, :],
                                     start=(h == 0), stop=(h == 2))
            u1m = sb.tile([128, 4, 64], BF16, tag="u1m")
            nc.vector.tensor_tensor(
                out=u1m[:].rearrange("k t q -> k (t q)"), in0=u1_ps[:],
                in1=M1W[:].rearrange("k t q -> k (t q)"), op=ALU.mult)
            zm_ps = ps.tile([64, 64], F32, tag="sm", bufs=2)
            for t in range(4):
                nc.tensor.matmul(zm_ps[:], wE[:, t * 64:(t + 1) * 64],
                                 u1m[:, t, :], start=(t == 0), stop=(t == 3))
            z2m = sb.tile([64, 64], BF16, tag="z2m")
            nc.vector.tensor_tensor(out=z2m[:], in0=zm_ps[:], in1=m2[:],
                                    op=ALU.mult)
            if debug:
                zm_dbg = sb.tile([64, 64], F32, tag="zm_dbg")
                nc.vector.tensor_copy(out=zm_dbg[:], in_=z2m[:])
                nc.sync.dma_start(out=probe("z2m", [64, 64])[:], in_=zm_dbg[:])

            # ---- Hopfield #2 (unnormalized; host divides by row 64)
            ST2 = ps.tile([128, 256], F32, tag="big", bufs=2)
            for t in range(4):
                nc.tensor.matmul(ST2[:, t * 64:(t + 1) * 64], KTc[t], z2m[:],
                                 start=True, stop=True)
            att2 = sb.tile([128, 256], BF16, tag="att2")
            nc.scalar.activation(out=att2[:], in_=ST2[:], func=AF.Exp,
                                 bias=0.0, scale=BETA)
            q2_ps = ps.tile([65, 64], F32, tag="sm", bufs=2)
            for t in range(4):
                nc.tensor.matmul(q2_ps[:], KV[:, t, :],
                                 att2[:, t * 64:(t + 1) * 64],
                                 start=(t == 0), stop=(t == 3))
            out_sb = sb.tile([65, 64], F32, tag="out_sb")
            nc.vector.tensor_copy(out=out_sb[:], in_=q2_ps[:])
            nc.sync.dma_start(out=out_t[:], in_=out_sb[:])
    nc.compile()
    return nc


def _get_nc(debug=False):
    key = ("nc", debug)
    if key not in _CACHE:
        _CACHE[key] = _build_nc(debug)
    return _CACHE[key]


# ---------------------------------------------------------------- entry point
def _make_in_maps(inputs):
    x = np.asarray(inputs["x"], np.float32)
    shared = _host_prep(*[np.asarray(inputs[k], np.float32)
                          for k in ("w1", "b1", "w2", "b2", "K", "Vw")])
    w1f, b1c = shared.pop("_w1f"), shared.pop("_b1")
    wC1, w2ta = shared.pop("_wC1"), shared.pop("_w2ta")
    bsz = x.shape[0]
    smpls = [_sample_prep(x[b], w1f, b1c, w2ta, wC1) for b in range(bsz)]
    in_maps = []
    for core in range(N_CORES):
        cvb, wc1b = smpls[core] if core < bsz else smpls[0]
        m = dict(shared)
        m["cv"], m["wC1"] = cvb, wc1b
        in_maps.append(m)
    return in_maps


def kernel(x, w1, b1, w2, b2, K, Vw, _debug=False):
    x = np.asarray(x, np.float32)
    bsz = x.shape[0]
    nc = _get_nc(_debug)
    in_maps = _make_in_maps(dict(x=x, w1=w1, b1=b1, w2=w2, b2=b2, K=K, Vw=Vw))
    res = run_bass_kernel_spmd(nc, in_maps, core_ids=list(range(N_CORES)))
    outs = []
    for b in range(bsz):
        q2 = np.asarray(res.results[b]["out"], np.float32)
        outs.append((q2[0:64] / q2[64:65]).reshape(64, 8, 8))
    out = np.stack(outs).astype(np.float32)
    if _debug:
        return out, res
    return out
